# revision 28
# baseline (speedup 1.0000x reference)
"""BatchConv2D (per-sample-weight conv) Trainium2 Bass kernel.

Problem: x [16,4,64,64,64], weight [16,128,64,3,3], bias [16,128] (all f32)
out[bi,bj] = conv2d(x[bi,bj], weight[bi], pad=1) + bias[bi]  -> [16,4,128,64,64]

Sharding: b_i axis split across 8 cores (2 per core); no communication.

Per-core kernel strategy ("parity-split dual-chain" conv-as-matmul, bf16):
  - Each image is stored in SBUF twice as [128 partitions, 33, 66]:
      img:     partitions 0..63 = EVEN padded rows, 64..127 = ODD padded rows
      imgSwap: the same data with the halves exchanged
    (parity slot r: even slot r = row 2r, odd slot r = row 2r-1; 1-pixel
    zero border baked in host-side). Both arrangements are pre-packed in
    DRAM host-side so each is ONE contiguous hardware-DGE DMA. The
    duplicate lets every conv tap of a given output-row parity be sourced
    from a single base partition:
      even outputs: ky=0/2 read img[64:], ky=1 reads imgSwap[64:]
      odd  outputs: ky=0/2 read img[:64], ky=1 reads imgSwap[:64]
  - Each 16-row output group runs as TWO 9-matmul K=64 PSUM chains (one
    per parity) in opposite PE row-halves (tile_position 64 / 0 — a
    chain's position must stay fixed, mixing positions within one PSUM
    accumulation chain wedges the PE). Interleaving them 1:1 keeps both
    halves streaming concurrently: 9 pair-slots x 512 rows per group.
  - fp8 DoubleRow (USE_F8): the two taps (ky=1,kx=0) and (ky=1,kx=2) are
    computed in ONE fp8-E4M3 DoubleRow matmul per chain (a custom rhs AP
    with a leading [2, stride 2] selector free-dim pairs the kx=0 and
    kx=2 column windows; wt8 holds the interleaved weight pair). This
    cuts the per-group schedule from 9 to 8 slots with the DR slot
    running at the fp8 double rate. The fp8 operand copies (swp8 / wt8)
    are quantized host-side; fp8 x fp8 products are exact in fp32 PSUM,
    so the measured rel-err 1.68e-2 is deterministic (harness gate 2e-2;
    bf16-only fallback USE_F8=False measures 3.5e-3).
  - Eviction splits across engines: DVE adds bias to the even-parity bank
    (TensorScalarPtr), the Activation engine does the odd bank (Identity
    activation with a per-partition bias AP). Both land in one per-image
    SBUF staging tile, stored by a single output DMA per image.
  - A 56-matmul PE warmup chain ("warm") streams during the initial input
    DMAs so the PE p-state is ramped before the first real matmul.
  - bf16 inputs AND outputs; the host casts the output back to f32.

Failed directions (measured, keep for posterity): K=128 ky-pairing (two
co-streamed opposite-half K=64 matmuls already cost the same as one
K=128 — and mixing tile_positions within one PSUM chain wedges the PE);
matmul outputs spanning 2 PSUM banks (ISA check rejects >512 f32);
column-split single-copy layout (16x32-element rhs runs read slower than
8x64, outweighing the 21% DMA-byte saving); eviction via the Pool engine
(no PSUM access); output-store queue moves, extra PSUM/SBUF buffering
(all flat).

Bench-only knobs (skip=...): idma/dve/odma ablate input-DMA / evictions /
output-DMA; cs = column-split layout; f8dr/warm select the fp8 DoubleRow
schedule and PE warmup (both ON in the real build via _build()).
"""

import numpy as np

B_I, B_J, C, H, W = 16, 4, 64, 64, 64
OC, KH, KW = 128, 3, 3
N_CORES = 8
BPC = B_I // N_CORES          # b_i per core
NIMG = BPC * B_J              # images per core
RH = H // 2 + 1               # 33 rows per parity half (padded)
WP = W + 2                    # 66 padded width
GROUPS = 4                    # output row-groups of 16 rows per image
RG = H // GROUPS // 2         # 8 output rows per parity per group

_CACHE = {}


def _build_nc(repeat=1, hw_loop=None, bench_out=False, skip=()):
    """hw_loop/bench_out/skip are bench-only knobs (unused by kernel())."""
    import concourse.mybir as mybir
    from concourse import bacc, tile

    F32 = mybir.dt.float32
    BF16 = mybir.dt.bfloat16
    COPY = mybir.ActivationFunctionType.Identity

    rg16 = "rg16" in skip
    rg4 = "rg4" in skip
    f8dr = "f8dr" in skip
    if rg16:
        ngroups, rg = GROUPS // 2, 2 * RG
    elif rg4:
        ngroups, rg = GROUPS * 2, RG // 2
    else:
        ngroups, rg = GROUPS, RG
    F8 = mybir.dt.float8e4
    xshape = [NIMG, 2, 2 * C, RH, WP]
    nc = bacc.Bacc("TRN2", target_bir_lowering=False, debug=False)
    if bench_out:
        x_d = nc.dram_tensor("x", xshape, BF16, kind="Internal")
        wt_d = nc.dram_tensor(
            "wt", [BPC, 2 * C, KH * KW, OC], BF16, kind="Internal"
        )
        b_d = nc.dram_tensor("bias", [OC, BPC], F32, kind="ExternalInput")
        o_d = nc.dram_tensor("oscr", [BPC, B_J, OC, H, W], BF16, kind="Internal")
        os_d = nc.dram_tensor("out", [OC, BPC], F32, kind="ExternalOutput")
        if f8dr:
            x8_d = nc.dram_tensor("x8", [NIMG, 2 * C, RH, WP], F8, kind="Internal")
            wt8_d = nc.dram_tensor("wt8", [BPC, 2 * C, 2, OC], F8, kind="Internal")
    else:
        x_d = nc.dram_tensor("x", xshape, BF16, kind="ExternalInput")
        wt_d = nc.dram_tensor(
            "wt", [BPC, 2 * C, KH * KW, OC], BF16, kind="ExternalInput"
        )
        b_d = nc.dram_tensor("bias", [OC, BPC], F32, kind="ExternalInput")
        o_d = nc.dram_tensor("out", [BPC, B_J, OC, H, W], BF16, kind="ExternalOutput")
        if f8dr:
            x8_d = nc.dram_tensor(
                "x8", [NIMG, 2 * C, RH, WP], F8, kind="ExternalInput"
            )
            wt8_d = nc.dram_tensor(
                "wt8", [BPC, 2 * C, 2, OC], F8, kind="ExternalInput"
            )


    with tile.TileContext(nc) as tc:
        with (
            tc.tile_pool(name="const", bufs=1) as cpool,
            tc.tile_pool(name="img", bufs=1) as ipool,
            tc.tile_pool(name="osb", bufs=1) as opool,
            tc.tile_pool(name="ps", bufs=1, space="PSUM") as pspool,
        ):
            wt_t = []
            wt8_t = []
            for bi in range(BPC):
                w = cpool.tile([2 * C, KH * KW, OC], BF16, name=f"wt{bi}", tag=f"wt{bi}")
                nc.sync.dma_start(w[:, :, :], wt_d[bi])
                wt_t.append(w)
                if f8dr:
                    w8 = cpool.tile([2 * C, 2, OC], F8, name=f"wt8_{bi}", tag=f"wt8_{bi}")
                    nc.sync.dma_start(w8[:, :, :], wt8_d[bi])
                    wt8_t.append(w8)
            bias_t = cpool.tile([OC, BPC], F32, name="bias_t", tag="bias")
            nc.sync.dma_start(bias_t[:, :], b_d[:, :])

            if "warm" in skip:
                # PE p-state warmup: stream small matmuls while the first
                # image DMAs land, so real matmuls start at full clock.
                wps = pspool.tile([OC, W], F32, name="wps", tag="wps")
                for _ in range(56):
                    nc.tensor.matmul(
                        wps[:, :],
                        wt_t[0][0:C, 0, :],
                        wt_t[0][0:C, 0, 0:W],
                        start=True,
                        stop=True,
                    )

            if "idma" in skip:
                img_c = cpool.tile([2 * C, RH, WP], BF16, name="imgc", tag="imgc")
                nc.sync.dma_start(img_c[:, :, :], x_d[0, 0])
                swp_c = cpool.tile([2 * C, RH, WP], BF16, name="swpc", tag="swpc")
                nc.sync.dma_start(swp_c[:, :, :], x_d[0, 1])
                if f8dr:
                    swp8_c = cpool.tile([2 * C, RH, WP], F8, name="swp8c", tag="swp8c")
                    nc.sync.dma_start(swp8_c[:, :, :], x8_d[0])

            if hw_loop is not None:
                loop_cm = tc.For_i(0, hw_loop, 1, name="rep")
                loop_cm.__enter__()

            for rep in range(repeat):
              for bi in range(BPC):
                for bj in range(B_J):
                    idx = bi * B_J + bj
                    ibufs = 8 if "bufs8" in skip else 4
                    if "idma" in skip:
                        img, swp = img_c, swp_c
                        if f8dr:
                            swp8 = swp8_c
                    else:
                        img = ipool.tile(
                            [2 * C, RH, WP], BF16, name="img", tag="img", bufs=ibufs
                        )
                        swp = ipool.tile(
                            [2 * C, RH, WP], BF16, name="swp", tag="swp", bufs=ibufs
                        )
                        nc.sync.dma_start(img[:, :, :], x_d[idx, 0])
                        nc.scalar.dma_start(swp[:, :, :], x_d[idx, 1])
                        if f8dr:
                            swp8 = ipool.tile(
                                [2 * C, RH, WP], F8, name="swp8", tag="swp8",
                                bufs=ibufs,
                            )
                            nc.scalar.dma_start(swp8[:, :, :], x8_d[idx])

                    osb = opool.tile(
                        [OC, GROUPS, RG, 2, W], BF16, name="osb", tag="osb",
                        bufs=3 if "osb3" in skip else 2,
                    )

                    for g in range(ngroups):
                        r0 = g * rg
                        psb = 2 if rg16 else (4 if "psb4" in skip else 3)
                        pst = {
                            k: pspool.tile(
                                [OC, rg, W], F32, name=f"ps{k}", tag=f"ps{k}",
                                bufs=psb,
                            )
                            for k in "AC"
                        }

                        def mm(bank, q, ky, kx, start, stop):
                            s = q + ky - 1
                            src = swp if ky == 1 else img
                            base = 64 * (1 - q)
                            rh0 = r0 + (1 if s >= 1 else 0)
                            wslot = 0 if "wsame" in skip else KW * ky + kx
                            if "flat" in skip:
                                rhs = src.rearrange("p a b -> p (a b)")[
                                    base : base + 64, kx : kx + rg * W
                                ]
                            else:
                                rhs = src[
                                    base : base + 64,
                                    rh0 : rh0 + rg,
                                    kx : kx + W,
                                ]
                            nc.tensor.matmul(
                                pst[bank][:, :, :],
                                wt_t[bi][base : base + 64, wslot, :],
                                rhs,
                                start=start,
                                stop=stop,
                            )

                        def mm_dr(bank, q, start, stop):
                            # fp8 DoubleRow: taps (ky=1,kx=0)+(ky=1,kx=2)
                            # in one matmul via a (2,stride2) selector dim
                            import bass_rust
                            base = 64 * (1 - q)
                            rh0 = r0 + q
                            s2 = swp8[base : base + 64, rh0 : rh0 + rg, 0:W]
                            rhs = s2.copy()
                            rhs.ap = bass_rust.VecI64Pair(
                                [tuple(s2.ap[0]), (2, 2), (WP, rg), (1, W)]
                            )
                            nc.tensor.matmul(
                                pst[bank][:, :, :],
                                wt8_t[bi][base : base + 64, :, :],
                                rhs,
                                start=start,
                                stop=stop,
                                perf_mode=mybir.MatmulPerfMode.DoubleRow,
                            )

                        if f8dr:
                            taps = [(0, 0), (0, 1), (0, 2), ("dr", None),
                                    (1, 1), (2, 0), (2, 1), (2, 2)]
                        else:
                            kys = (0, 2, 1) if "tapord" in skip else (0, 1, 2)
                            taps = [(ky, kx) for ky in kys for kx in range(KW)]
                        tmax = len(taps) - 1
                        for t, (ky, kx) in enumerate(taps):
                            if ky == "dr":
                                mm_dr("A", 0, start=(t == 0), stop=(t == tmax))
                                mm_dr("C", 1, start=(t == 0), stop=(t == tmax))
                            else:
                                mm("A", 0, ky, kx, start=(t == 0), stop=(t == tmax))
                                mm("C", 1, ky, kx, start=(t == 0), stop=(t == tmax))

                        if "dve" in skip:
                            continue
                        if rg16:
                            oA = osb[:, 2 * g : 2 * g + 2, :, 0, :]
                            oC = osb[:, 2 * g : 2 * g + 2, :, 1, :]
                        else:
                            oA = osb[:, g, :, 0, :]
                            oC = osb[:, g, :, 1, :]
                        nc.vector.tensor_scalar_add(
                            oA, pst["A"][:, :, :], bias_t[:, bi : bi + 1],
                        )
                        nc.scalar.activation(
                            oC, pst["C"][:, :, :],
                            COPY, bias=bias_t[:, bi : bi + 1],
                        )

                    if "dve" in skip or "odma" in skip:
                        continue
                    if "og4" in skip:
                        for g in range(GROUPS):
                            oq = nc.sync if g % 2 == 0 else nc.scalar
                            oq.dma_start(
                                o_d[bi, bj, :, 16 * g : 16 * g + 16, :],
                                osb[:, g, :, :, :],
                            )
                    else:
                        # SP queue: the Act queue already issues the swp
                        # loads and all evictions
                        oq = nc.gpsimd if "oq" in skip else nc.sync
                        oq.dma_start(o_d[bi, bj], osb[:, :, :, :, :])

            if hw_loop is not None:
                loop_cm.__exit__(None, None, None)
            if bench_out:
                nc.sync.dma_start(os_d[:, :], bias_t[:, :])
    nc.compile()
    return nc


USE_F8 = True  # fp8 DoubleRow for the (ky=1, kx=0/2) taps
USE_CS = False  # column-split single-copy layout

HP = H + 2   # padded rows (col-split layout)
WC = W // 2 + 2  # cols per half + halo
RGC = H // GROUPS  # 16 output rows per group (col-split)


def _build_nc_cs(hw_loop=None, bench_out=False, skip=()):
    """Column-split layout: partitions 0..63 = 64 channels of the LEFT
    half-image (padded cols -1..32), 64..127 = RIGHT half (cols 31..64),
    all 66 padded rows. One x copy (plus an fp8 one for the DoubleRow
    taps) instead of img+swp. Chain L (output cols 0..31) reads only
    partitions 0..63 (PE position 0), chain R (cols 32..63) reads
    64..127 (position 64); same 1:1 co-stream slot schedule.
    """
    import concourse.mybir as mybir
    from concourse import bacc, tile
    import bass_rust

    F32 = mybir.dt.float32
    BF16 = mybir.dt.bfloat16
    F8 = mybir.dt.float8e4
    COPY = mybir.ActivationFunctionType.Identity
    f8dr = "f8dr" in skip
    WH = W // 2

    nc = bacc.Bacc("TRN2", target_bir_lowering=False, debug=False)
    kind_x = "Internal" if bench_out else "ExternalInput"
    x_d = nc.dram_tensor("x", [NIMG, 2 * C, HP, WC], BF16, kind=kind_x)
    wt_d = nc.dram_tensor("wt", [BPC, 2 * C, KH * KW, OC], BF16, kind=kind_x)
    b_d = nc.dram_tensor("bias", [OC, BPC], F32, kind="ExternalInput")
    if f8dr:
        x8_d = nc.dram_tensor("x8", [NIMG, 2 * C, HP, WC], F8, kind=kind_x)
        wt8_d = nc.dram_tensor("wt8", [BPC, 2 * C, 2, OC], F8, kind=kind_x)
    if bench_out:
        o_d = nc.dram_tensor("oscr", [BPC, B_J, OC, H, W], BF16, kind="Internal")
        os_d = nc.dram_tensor("out", [OC, BPC], F32, kind="ExternalOutput")
    else:
        o_d = nc.dram_tensor("out", [BPC, B_J, OC, H, W], BF16, kind="ExternalOutput")

    with tile.TileContext(nc) as tc:
        with (
            tc.tile_pool(name="const", bufs=1) as cpool,
            tc.tile_pool(name="img", bufs=1) as ipool,
            tc.tile_pool(name="osb", bufs=1) as opool,
            tc.tile_pool(name="ps", bufs=1, space="PSUM") as pspool,
        ):
            wt_t, wt8_t = [], []
            for bi in range(BPC):
                w = cpool.tile([2 * C, KH * KW, OC], BF16, name=f"wt{bi}", tag=f"wt{bi}")
                nc.sync.dma_start(w[:, :, :], wt_d[bi])
                wt_t.append(w)
                if f8dr:
                    w8 = cpool.tile([2 * C, 2, OC], F8, name=f"wt8_{bi}", tag=f"wt8_{bi}")
                    nc.sync.dma_start(w8[:, :, :], wt8_d[bi])
                    wt8_t.append(w8)
            bias_t = cpool.tile([OC, BPC], F32, name="bias_t", tag="bias")
            nc.sync.dma_start(bias_t[:, :], b_d[:, :])

            if "warm" in skip:
                wps = pspool.tile([OC, W], F32, name="wps", tag="wps")
                for _ in range(56):
                    nc.tensor.matmul(
                        wps[:, :], wt_t[0][0:C, 0, :], wt_t[0][0:C, 0, 0:W],
                        start=True, stop=True,
                    )

            if hw_loop is not None:
                loop_cm = tc.For_i(0, hw_loop, 1, name="rep")
                loop_cm.__enter__()

            for bi in range(BPC):
                for bj in range(B_J):
                    idx = bi * B_J + bj
                    ibufs = 8 if "bufs8" in skip else 4
                    xc = ipool.tile(
                        [2 * C, HP, WC], BF16, name="xc", tag="xc", bufs=ibufs
                    )
                    nc.sync.dma_start(xc[:, :, :], x_d[idx])
                    if f8dr:
                        xc8 = ipool.tile(
                            [2 * C, HP, WC], F8, name="xc8", tag="xc8", bufs=ibufs
                        )
                        nc.scalar.dma_start(xc8[:, :, :], x8_d[idx])

                    osb = opool.tile(
                        [OC, GROUPS, RGC, 2, WH], BF16, name="osb", tag="osb",
                        bufs=3 if "osb3" in skip else 2,
                    )

                    for g in range(GROUPS):
                        r0 = g * RGC
                        psb = 4 if "psb4" in skip else 3
                        pst = {
                            k: pspool.tile(
                                [OC, RGC, WH], F32, name=f"ps{k}", tag=f"ps{k}",
                                bufs=psb,
                            )
                            for k in "LR"
                        }

                        def mm(bank, ky, kx, start, stop):
                            base = 0 if bank == "L" else 64
                            nc.tensor.matmul(
                                pst[bank][:, :, :],
                                wt_t[bi][base : base + 64, KW * ky + kx, :],
                                xc[
                                    base : base + 64,
                                    r0 + ky : r0 + ky + RGC,
                                    kx : kx + WH,
                                ],
                                start=start,
                                stop=stop,
                            )

                        def mm_dr(bank, start, stop):
                            base = 0 if bank == "L" else 64
                            s2 = xc8[
                                base : base + 64, r0 + 1 : r0 + 1 + RGC, 0:WH
                            ]
                            rhs = s2.copy()
                            rhs.ap = bass_rust.VecI64Pair(
                                [tuple(s2.ap[0]), (2, 2), (WC, RGC), (1, WH)]
                            )
                            nc.tensor.matmul(
                                pst[bank][:, :, :],
                                wt8_t[bi][base : base + 64, :, :],
                                rhs,
                                start=start,
                                stop=stop,
                                perf_mode=mybir.MatmulPerfMode.DoubleRow,
                            )

                        if f8dr:
                            taps = [(0, 0), (0, 1), (0, 2), ("dr", None),
                                    (1, 1), (2, 0), (2, 1), (2, 2)]
                        else:
                            taps = [(ky, kx) for ky in (0, 1, 2)
                                    for kx in range(KW)]
                        tmax = len(taps) - 1
                        for t, (ky, kx) in enumerate(taps):
                            if ky == "dr":
                                mm_dr("L", start=(t == 0), stop=(t == tmax))
                                mm_dr("R", start=(t == 0), stop=(t == tmax))
                            else:
                                mm("L", ky, kx, start=(t == 0), stop=(t == tmax))
                                mm("R", ky, kx, start=(t == 0), stop=(t == tmax))

                        if "dve" in skip:
                            continue
                        nc.vector.tensor_scalar_add(
                            osb[:, g, :, 0, :], pst["L"][:, :, :],
                            bias_t[:, bi : bi + 1],
                        )
                        nc.scalar.activation(
                            osb[:, g, :, 1, :], pst["R"][:, :, :],
                            COPY, bias=bias_t[:, bi : bi + 1],
                        )

                    if "dve" in skip or "odma" in skip:
                        continue
                    oq = nc.gpsimd if "oq" in skip else nc.sync
                    oq.dma_start(o_d[bi, bj], osb[:, :, :, :, :])

            if hw_loop is not None:
                loop_cm.__exit__(None, None, None)
            if bench_out:
                nc.sync.dma_start(os_d[:, :], bias_t[:, :])
    nc.compile()
    return nc


def _pack(x, weight, bias):
    """Host-side repack into the kernel's DMA-friendly layouts."""
    import ml_dtypes

    bf16 = ml_dtypes.bfloat16
    x = np.ascontiguousarray(x, dtype=np.float32).astype(bf16)
    wq = np.ascontiguousarray(weight, dtype=np.float32).astype(bf16)
    bias = np.ascontiguousarray(bias, dtype=np.float32)

    if USE_CS:
        xq = np.zeros((B_I, B_J, 2, C, HP, WC), bf16)
        # half 0: padded global cols -1..32; half 1: cols 31..64
        xq[:, :, 0, :, 1 : H + 1, 1:WC] = x[:, :, :, :, 0 : WC - 1]
        xq[:, :, 1, :, 1 : H + 1, 0 : WC - 1] = x[:, :, :, :, W // 2 - 1 :]
        xq = xq.reshape(B_I, B_J, 2 * C, HP, WC)

        wt0 = np.ascontiguousarray(np.transpose(wq, (0, 2, 3, 4, 1))).reshape(
            B_I, C, KH * KW, OC
        )
        wt = np.concatenate([wt0, wt0], axis=1)
        bp = np.ascontiguousarray(np.transpose(bias, (1, 0)))
        if not USE_F8:
            return xq, wt, bp
        f8 = ml_dtypes.float8_e4m3fn
        x8 = xq.astype(f8)
        t10 = np.transpose(wq[:, :, :, 1, 0], (0, 2, 1))
        t12 = np.transpose(wq[:, :, :, 1, 2], (0, 2, 1))
        wt8 = np.stack([t10, t12], axis=2)
        wt8 = np.concatenate([wt8, wt8], axis=1).astype(f8)
        return xq, wt, bp, x8, wt8

    xp = np.zeros((B_I, B_J, 2, C, RH, WP), bf16)
    xp[:, :, 0, :, 0:32, 1 : W + 1] = x[:, :, :, 0::2, :]   # even slot r = row 2r
    xp[:, :, 1, :, 1:33, 1 : W + 1] = x[:, :, :, 1::2, :]   # odd slot r = row 2r-1
    xp = xp.reshape(B_I, B_J, 2 * C, RH, WP)
    # second copy with the partition halves pre-swapped (the swp-tile load)
    xp = np.stack(
        [xp, np.concatenate([xp[:, :, C:], xp[:, :, :C]], axis=2)], axis=2
    )

    wt0 = np.ascontiguousarray(np.transpose(wq, (0, 2, 3, 4, 1))).reshape(
        B_I, C, KH * KW, OC
    )
    wt = np.concatenate([wt0, wt0], axis=1)  # duplicate across partition halves

    bp = np.ascontiguousarray(np.transpose(bias, (1, 0)))  # [OC, B_I]
    if not USE_F8:
        return xp, wt, bp

    f8 = ml_dtypes.float8_e4m3fn
    x8 = xp[:, :, 1].astype(f8)  # the swp arrangement, fp8
    # wt8 [B_I, 2C, 2, OC]: slot 0 = w(ky=1,kx=0), slot 1 = w(ky=1,kx=2),
    # duplicated across partition halves
    t10 = np.transpose(wq[:, :, :, 1, 0], (0, 2, 1))  # [B_I, C, OC]
    t12 = np.transpose(wq[:, :, :, 1, 2], (0, 2, 1))
    wt8 = np.stack([t10, t12], axis=2)               # [B_I, C, 2, OC]
    wt8 = np.concatenate([wt8, wt8], axis=1).astype(f8)
    return xp, wt, bp, x8, wt8


def make_in_maps(xp, wt, bp, x8=None, wt8=None):
    in_maps = []
    for i in range(N_CORES):
        sl = slice(i * BPC, (i + 1) * BPC)
        m = {
            "x": np.ascontiguousarray(xp[sl].reshape(NIMG, *xp.shape[2:])),
            "wt": np.ascontiguousarray(wt[sl]),
            "bias": np.ascontiguousarray(bp[:, sl]),
        }
        if x8 is not None:
            m["x8"] = np.ascontiguousarray(
                x8[sl].reshape(NIMG, *x8.shape[2:])
            )
            m["wt8"] = np.ascontiguousarray(wt8[sl])
        in_maps.append(m)
    return in_maps


def _build(hw_loop=None, bench_out=False, skip=None):
    if skip is None:
        skip = ("f8dr", "warm") if USE_F8 else ("warm",)
    builder = _build_nc_cs if USE_CS else _build_nc
    return builder(hw_loop=hw_loop, bench_out=bench_out, skip=skip)


def kernel(x, weight, bias):
    from concourse.bass_utils import run_bass_kernel_spmd

    packs = _pack(x, weight, bias)

    if "nc" not in _CACHE:
        _CACHE["nc"] = _build()
    nc = _CACHE["nc"]

    in_maps = make_in_maps(*packs)

    res = run_bass_kernel_spmd(nc, in_maps, list(range(N_CORES)))
    out = np.concatenate(
        [res.results[i]["out"].astype(np.float32) for i in range(N_CORES)], axis=0
    )
    return out
